# revision 8
# baseline (speedup 1.0000x reference)
"""GQA attention kernel for Trainium2, tensor-parallel over (batch, kv-head-pair).

Problem: B=2, S=2048, D=2048, 32 q heads / 8 kv heads, head_dim 64,
scores get an additive mask [1,1,S,S] + per-batch graph bias [B,1,S,S].

Sharding: 16 units = (batch 2) x (kv-head-pair 4) over 8 cores; core c handles
batch b = c % 2 and kv heads {2*(c//2), 2*(c//2)+1} (8 q heads). Each core
computes its heads' attention output and its slice of the wo matmul; the host
sums the 4 partial outputs per batch.

The execution environment prices each TensorE instruction at a large flat cost
while VectorE/ScalarE/DMA work is comparatively free, so the kernel is shaped
to minimize matmul-instruction count and keep the PE stream unblocked:
  - x arrives bf16 and is transposed by the DMA xbar (no PE transposes).
  - QKV projections run in bf16 (same MM count, enables the DMA transpose).
  - scoresT = xkT-tile.T @ xqT in fp32, [kpos, (rep, q)] layout, so the
    probabilities are already transposed for the PV matmul - no probs
    transpose anywhere.
  - probsT = exp(s/8) * exp(mask+bias); exp on ScalarE (free scale=1/8
    affine), bias pre-combined on host, its exp done once on device.
  - attnT = [xv|1].T @ probsT accumulates over kpos tiles; the ones column
    makes row 64 the softmax denominator, normalized afterwards on VectorE.
  - y = attnT-tile.T @ wo in fp32.
The score->exp->mul->PV chain is software-pipelined (lookahead 2) so the PE
never waits on ScalarE/VectorE.
"""

import sys

if "/opt/trn_rl_repo" not in sys.path:
    sys.path.insert(0, "/opt/trn_rl_repo")

import numpy as np
import ml_dtypes
from contextlib import ExitStack

import concourse.bass as bass
import concourse.tile as tile
from concourse import bacc, mybir
from concourse.bass_utils import run_bass_kernel_spmd

F32 = mybir.dt.float32
BF16 = mybir.dt.bfloat16

D = 2048          # model dim
HD = 64           # head dim
NREP = 4          # q heads per kv head
NKVL = 2          # kv heads per core
N_CORES = 8
DOUT_Q = NREP * NKVL * HD   # 512
WCOLS = DOUT_Q + 2 * NKVL * HD  # 768


def build_program(S=2048, causal=False, loop_n=1):
    G = S // 128   # q groups
    TK = S // 128  # kpos tiles
    assert S % 512 == 0

    nc = bacc.Bacc("TRN2", target_bir_lowering=False, debug=False,
                   num_devices=N_CORES)
    x_d = nc.dram_tensor("x", (S, D), BF16, kind="ExternalInput").ap()
    comb_d = nc.dram_tensor("comb", (S, S), BF16, kind="ExternalInput").ap()
    wqkv_d = nc.dram_tensor("wqkv", (D, WCOLS), BF16, kind="ExternalInput").ap()
    wo_d = nc.dram_tensor("wo", (DOUT_Q, D), F32, kind="ExternalInput").ap()
    vt_d = nc.dram_tensor("vt_scratch", (128, S), BF16, kind="Internal").ap()
    y_d = nc.dram_tensor("y", (S, D), F32, kind="ExternalOutput").ap()

    with tile.TileContext(nc) as tc, ExitStack() as ctx:
        def body():
            with ExitStack() as bctx:
                persist = bctx.enter_context(tc.tile_pool(name="persist", bufs=1))
                xqT = persist.tile([128, NREP * S], F32)      # [(kvl,d), (rep,q)]
                xkT = persist.tile([128, S], F32)             # [(kvl,d), kpos]
                xve = persist.tile([128, NKVL * TK * 65], F32)  # [kpos,(kvl,tk,d+1)]
                attnT = persist.tile([128, NREP * S], F32)    # [(kvl,d), (rep,q)]
                xqT3 = xqT.rearrange("p (h q) -> p h q", h=NREP)
                attnT3 = attnT.rearrange("p (h q) -> p h q", h=NREP)
                xve4 = xve.rearrange("p (v t c) -> p v t c", v=NKVL, c=65)
                nc.vector.memset(xve4[:, :, :, 64:65], 1.0)

                # ---------------- Phase A: projections ----------------
                with tc.tile_pool(name="xt_pool", bufs=1) as xtp, \
                     tc.tile_pool(name="wq_pool", bufs=1) as wpool, \
                     tc.tile_pool(name="vfix_pool", bufs=2) as vfp, \
                     tc.tile_pool(name="psB", bufs=1, space="PSUM") as psB:
                    w_sb = wpool.tile([128, 16 * WCOLS], BF16)
                    w3 = w_sb.rearrange("p (t o) -> p t o", t=16)
                    nc.sync.dma_start(w3, wqkv_d.rearrange("(t p) o -> p t o", p=128))
                    xT = xtp.tile([128, 16 * S], BF16)
                    xT3 = xT.rearrange("p (t s) -> p t s", t=16)
                    for tin in range(16):
                        nc.sync.dma_start_transpose(
                            xT3[:, tin, :], x_d[:, tin * 128:(tin + 1) * 128])

                    NSC = S // 512
                    for sc in range(NSC):
                        psQ = [psB.tile([128, 512], F32, tag=f"psq{r}",
                                        name=f"psq{r}") for r in range(NREP)]
                        psK = psB.tile([128, 512], F32, tag="psk")
                        psV = psB.tile([128, 512], F32, tag="psv")
                        for tin in range(16):
                            rhs = xT3[:, tin, sc * 512:(sc + 1) * 512]
                            for r in range(NREP):
                                nc.tensor.matmul(psQ[r], w3[:, tin, r * 128:(r + 1) * 128],
                                                 rhs, start=(tin == 0), stop=(tin == 15))
                            nc.tensor.matmul(psK, w3[:, tin, 512:640], rhs,
                                             start=(tin == 0), stop=(tin == 15))
                            nc.tensor.matmul(psV, w3[:, tin, 640:768], rhs,
                                             start=(tin == 0), stop=(tin == 15))
                        for r in range(NREP):
                            nc.scalar.copy(xqT3[:, r, sc * 512:(sc + 1) * 512], psQ[r])
                        nc.scalar.copy(xkT[:, sc * 512:(sc + 1) * 512], psK)
                        vts = vfp.tile([128, 512], BF16, tag="vts")
                        nc.vector.tensor_copy(vts, psV)
                        nc.sync.dma_start(vt_d[:, sc * 512:(sc + 1) * 512], vts)
                    # transpose V back: [128 (kvl,d), S] -> per-tile [128 s, 128]
                    for tk in range(TK):
                        vn = vfp.tile([128, 128], BF16, tag="vn", bufs=3, name="vn")
                        nc.sync.dma_start_transpose(vn, vt_d[:, tk * 128:(tk + 1) * 128])
                        nc.vector.tensor_copy(xve4[:, 0, tk, 0:64], vn[:, 0:64])
                        nc.vector.tensor_copy(xve4[:, 1, tk, 0:64], vn[:, 64:128])

                # ---------------- Phase B: attention ----------------
                with tc.tile_pool(name="expCT_pool", bufs=1) as ecp, \
                     tc.tile_pool(name="combT_pool", bufs=2) as ctp, \
                     tc.tile_pool(name="eS_pool", bufs=4) as esp, \
                     tc.tile_pool(name="eT_pool", bufs=4) as etp, \
                     tc.tile_pool(name="norm_pool", bufs=4) as nrm, \
                     tc.tile_pool(name="psS", bufs=4, space="PSUM") as psS, \
                     tc.tile_pool(name="psO", bufs=4, space="PSUM") as psO:
                    expCT = ecp.tile([128, TK * S], BF16)
                    expCT3 = expCT.rearrange("p (t q) -> p t q", t=TK)
                    for t in range(TK):
                        combT = ctp.tile([128, S], BF16, tag="combT")
                        nc.sync.dma_start_transpose(combT, comb_d[:, t * 128:(t + 1) * 128])
                        nc.scalar.activation(expCT3[:, t, :], combT,
                                             mybir.ActivationFunctionType.Exp)

                    for g in range(G):
                        tmax = min(g + 1, TK) if causal else TK
                        oP = [psO.tile([128, 512], F32, tag="po", name=f"po{kvl}")
                              for kvl in range(NKVL)]

                        eTq = []  # pending (t, [eT_kv0, eT_kv1])

                        def emit_score(t, g=g):
                            eTs = []
                            for kvl in range(NKVL):
                                p0, p1 = kvl * 64, (kvl + 1) * 64
                                sS = psS.tile([128, 512], F32, tag="ps",
                                              name=f"ps{kvl}")
                                nc.tensor.matmul(
                                    sS, xkT[p0:p1, t * 128:(t + 1) * 128],
                                    xqT3[p0:p1, :, g * 128:(g + 1) * 128],
                                    start=True, stop=True)
                                eS = esp.tile([128, 512], F32, tag="eS", name="eS")
                                nc.scalar.activation(eS, sS,
                                                     mybir.ActivationFunctionType.Exp,
                                                     scale=0.125)
                                eT = etp.tile([128, 512], F32, tag="eT", name="eT")
                                in1 = (expCT3[:, t:t + 1, g * 128:(g + 1) * 128]
                                       .unsqueeze(2).broadcast_to((128, 1, NREP, 128)))
                                nc.vector.tensor_mul(
                                    eT.rearrange("p (o h q) -> p o h q", o=1, h=NREP),
                                    eS.rearrange("p (o h q) -> p o h q", o=1, h=NREP),
                                    in1)
                                eTs.append(eT)
                            eTq.append((t, eTs))

                        def emit_pv(tmax=tmax, oP=oP):
                            t, eTs = eTq.pop(0)
                            for kvl in range(NKVL):
                                nc.tensor.matmul(
                                    oP[kvl][0:65, :], xve4[:, kvl, t, :], eTs[kvl],
                                    start=(t == 0), stop=(t == tmax - 1))

                        for t in range(tmax):
                            emit_score(t)
                            if t >= 2:
                                emit_pv()
                        while eTq:
                            emit_pv()

                        for kvl in range(NKVL):
                            ssum = nrm.tile([1, 512], F32, tag="ssum", name="ssum")
                            nc.vector.tensor_scalar_add(ssum, oP[kvl][64:65, :], 1e-30)
                            rec = nrm.tile([1, 512], F32, tag="rec", name="rec")
                            nc.vector.reciprocal(rec, ssum)
                            recb = nrm.tile([64, 512], F32, tag="recb", name="recb")
                            nc.gpsimd.partition_broadcast(recb, rec)
                            rec_b = recb.rearrange("p (h q) -> p h q", h=NREP)
                            src = oP[kvl][0:64, :].rearrange("p (h q) -> p h q", h=NREP)
                            if kvl == 0:
                                nc.vector.tensor_mul(
                                    attnT3[0:64, :, g * 128:(g + 1) * 128], src, rec_b)
                            else:
                                shift = nrm.tile([64, 512], F32, tag="shift",
                                                 name="shift")
                                nc.vector.tensor_mul(
                                    shift.rearrange("p (h q) -> p h q", h=NREP),
                                    src, rec_b)
                                nc.sync.dma_start(
                                    attnT3[64:128, :, g * 128:(g + 1) * 128],
                                    shift.rearrange("p (h q) -> p h q", h=NREP))

                # ---------------- Phase C: output projection ----------------
                with tc.tile_pool(name="wo_pool", bufs=1) as wop, \
                     tc.tile_pool(name="y_pool", bufs=3) as yp, \
                     tc.tile_pool(name="psY", bufs=2, space="PSUM") as psY:
                    wo_sb = wop.tile([128, NREP * D], F32)
                    wo3 = wo_sb.rearrange("p (r n) -> p r n", r=NREP)
                    nc.sync.dma_start(wo3, wo_d.rearrange("(r p) n -> p r n", p=128))
                    for st in range(S // 128):
                        pY = psY.tile([128, D], F32, tag="py", name="py")
                        for r in range(NREP):
                            lhsT = attnT3[:, r, st * 128:(st + 1) * 128]
                            for nch in range(4):
                                nc.tensor.matmul(pY[:, nch * 512:(nch + 1) * 512],
                                                 lhsT, wo3[:, r, nch * 512:(nch + 1) * 512],
                                                 start=(r == 0), stop=(r == NREP - 1))
                        y_sb = yp.tile([128, D], F32, tag="ysb", name="ysb")
                        if st % 2 == 0:
                            nc.vector.tensor_copy(y_sb, pY)
                        else:
                            nc.scalar.copy(y_sb, pY)
                        nc.sync.dma_start(y_d[st * 128:(st + 1) * 128, :], y_sb)

        for _rep in range(loop_n):
            body()

    nc.compile()
    return nc


def shard_inputs(x, mask, graph_bias, wq, wk, wv, wo, S=2048):
    """Build the 8 per-core input maps from the full inputs."""
    mask2 = np.asarray(mask, dtype=np.float32).reshape(S, S)
    gb = np.asarray(graph_bias, dtype=np.float32).reshape(2, S, S)
    comb_b = [(mask2 + gb[b]).astype(ml_dtypes.bfloat16) for b in range(2)]
    x = np.asarray(x, dtype=np.float32)
    x_bf = [np.ascontiguousarray(x[b]).astype(ml_dtypes.bfloat16) for b in range(2)]
    wq = np.asarray(wq, dtype=np.float32)
    wk = np.asarray(wk, dtype=np.float32)
    wv = np.asarray(wv, dtype=np.float32)
    wo = np.asarray(wo, dtype=np.float32)

    in_maps = []
    for c in range(N_CORES):
        b = c % 2
        kvp = c // 2
        kvg = (2 * kvp, 2 * kvp + 1)
        qcols, orows = [], []
        for r in range(NREP):
            for kv in kvg:
                h = kv * NREP + r
                qcols.extend(range(h * HD, (h + 1) * HD))
                orows.extend(range(h * HD, (h + 1) * HD))
        kcols = []
        for kv in kvg:
            kcols.extend(range(kv * HD, (kv + 1) * HD))
        wqkv = np.concatenate(
            [wq[:, qcols], wk[:, kcols], wv[:, kcols]], axis=1)
        in_maps.append({
            "x": x_bf[b],
            "comb": comb_b[b],
            "wqkv": np.ascontiguousarray(wqkv.astype(ml_dtypes.bfloat16)),
            "wo": np.ascontiguousarray(wo[orows, :]),
        })
    return in_maps


def gather_outputs(results, S=2048):
    y = np.zeros((2, S, D), dtype=np.float32)
    for c in range(N_CORES):
        y[c % 2] += results[c]["y"]
    return y


def detect_causal(mask, graph_bias, S=2048):
    """True if every score tile strictly above the block diagonal is fully
    masked (so the kernel may skip it): those tiles then contribute exactly 0
    probability, matching the reference."""
    if S % 128:
        return False
    m = np.asarray(mask, dtype=np.float32).reshape(S, S)
    nb = S // 128
    blockmax = m.reshape(nb, 128, nb, 128).max(axis=(1, 3))
    upper = np.triu(np.ones((nb, nb), dtype=bool), k=1)
    if not upper.any():
        return False
    if not bool((blockmax[upper] < -1e8).all()):
        return False
    return float(np.abs(np.asarray(graph_bias)).max()) < 1e6


_PROGRAM_CACHE = {}


def _get_program(S, causal, loop_n=1):
    key = (S, causal, loop_n)
    if key not in _PROGRAM_CACHE:
        _PROGRAM_CACHE[key] = build_program(S=S, causal=causal, loop_n=loop_n)
    return _PROGRAM_CACHE[key]


def kernel(x, mask, graph_bias, wq, wk, wv, wo, start_pos=0):
    S = x.shape[1]
    causal = detect_causal(mask, graph_bias, S=S)
    nc = _get_program(S, causal)
    in_maps = shard_inputs(x, mask, graph_bias, wq, wk, wv, wo, S=S)
    res = run_bass_kernel_spmd(nc, in_maps, core_ids=list(range(N_CORES)))
    return gather_outputs(res.results, S=S)
